# revision 13
# baseline (speedup 1.0000x reference)
"""Trainium2 Bass kernel for batched contrastive loss (InfoNCE over CxC sims).

Math (matches the jax reference):
    v_hat = v / ||v||,  t_hat = t / ||t||          (L2 over D, eps=1e-12)
    L[b,c,k] = (v_hat[b,c] . t_hat[b,k]) / 0.5     (logits)
    loss = mean_{b,c} [ logsumexp_k L[b,c,k] - L[b,c,c] ]

Strategy (8 NeuronCores, data-parallel over B=64 -> 8 batches/core).

v2 rebalance (from v1 trace: Vector 62us / Scalar 57us busy of an 89us span):
  - ONE fused elementwise stage: v*v, t*t and v*t all write one concatenated
    scratch [128, 24, 256]; a two-level halving tree (2x-mode tensor_adds)
    shrinks it 4x before the 1x-mode grouped reduce. Norms AND the positive
    dot products come out of a single [128, 24] reduce per pair.
  - Transposed Gram G'[k, c] = (Tt)^T @ Vhat_t: V is pre-scaled by
    sv=2/||v|| (per-partition bcast mult in natural layout), so the exp's
    per-partition scale slot carries tsc_k = 1/||t_k||. No T_hat tensor.
  - Row-sums over k are PARTITION sums in this orientation: a ones[128,1]
    matmul accumulates exp chunks into one PSUM bank (rows = batches).
    Kills all 32 ACT accumulator reads (~9.3us) of v1.
  - PSUM->SBUF transpose copies moved off Scalar to GpSimd/Pool (copy
    efficiency 0.6 vs multiply 0.42 -> copies are the one big op class
    Pool does ~as well as the busy engines).
  - rsqrt without Rsqrt and within ONE ACT table set (via the
    get_activation_tables patch): sv = exp(-0.5*ln(nv2/4)), tsc =
    exp(-0.5*ln(nt2)).
  - Output: per-core partial sums lossA[8,1] = sum_c ln(rowsum) per batch,
    lossB[128,1] = sum of positive logits; host combines.
"""

import math
from contextlib import ExitStack

import numpy as np

import concourse.bacc as bacc
import concourse.bass as bass
import concourse.tile as tile
from concourse import mybir
from concourse.bass_utils import run_bass_kernel_spmd
from concourse.masks import make_identity

N_CORES = 8
B_PER_CORE = 8
PB = 2  # batches per DMA pair
NPAIR = B_PER_CORE // PB
C = 512
D = 256
P = 128
NCHUNK = C // P  # 4 c-chunks per batch
NDHALF = D // P  # 2 d-halves
NG = 3 * PB * NCHUNK  # 24 reduce groups: [v*v | t*t | v*t] x (pb, chunk)

F32 = mybir.dt.float32
BF16 = mybir.dt.bfloat16

# ---------------------------------------------------------------------------
# Keep ACT on a single table set: exp & ln both live in
# "natural_log_exp_and_others"; by removing them from every other set, the
# insert_act_table_loads fixpoint must pick that one set for both, so the
# kernel pays ONE table load instead of thrashing (~1.3us per reload).
_orig_get_tables = bacc.get_activation_tables


def _patched_get_tables(arch):
    tables = dict(_orig_get_tables(arch))
    keep = "natural_log_exp_and_others"
    strip = {mybir.ActivationFunctionType.Exp, mybir.ActivationFunctionType.Ln}
    if keep in tables:
        for name in tables:
            if name != keep:
                tables[name] = set(tables[name]) - strip
    return tables


bacc.get_activation_tables = _patched_get_tables


def _bcast_cols(tile_ap, col0, ncols_outer, ncols_inner, bcast_count):
    """AP reading tile[:, col0 + o*ncols_inner + i] broadcast bcast_count
    times along a new innermost (stride-0) dim."""
    base = tile_ap[:, col0 : col0 + ncols_outer * ncols_inner]
    part_dim = base.ap[0]
    elem_step = base.ap[-1][0]
    return bass.AP(
        tensor=base.tensor,
        offset=base.offset,
        ap=[
            part_dim,
            [elem_step * ncols_inner, ncols_outer],
            [elem_step, ncols_inner],
            [0, bcast_count],
        ],
    )


def _emit(ctx: ExitStack, tc: tile.TileContext, lossA_ap, lossB_ap, v_ap, t_ap):
    nc = tc.nc
    ctx.enter_context(nc.allow_low_precision("bf16 norm/pos reduces, 2x DVE"))

    singles = ctx.enter_context(tc.tile_pool(name="singles", bufs=1))
    inputs = ctx.enter_context(tc.tile_pool(name="inputs", bufs=3))
    normed = ctx.enter_context(tc.tile_pool(name="normed", bufs=2))
    sqpool = ctx.enter_context(tc.tile_pool(name="sq", bufs=2))
    hpool = ctx.enter_context(tc.tile_pool(name="h", bufs=2))
    trans = ctx.enter_context(tc.tile_pool(name="trans", bufs=4))
    epool = ctx.enter_context(tc.tile_pool(name="E", bufs=6))
    stats = ctx.enter_context(tc.tile_pool(name="stats", bufs=2))
    tp_pool = ctx.enter_context(tc.tile_pool(name="tp", bufs=2, space="PSUM"))
    gp_pool = ctx.enter_context(tc.tile_pool(name="gp", bufs=3, space="PSUM"))
    rs_pool = ctx.enter_context(tc.tile_pool(name="rs", bufs=1, space="PSUM"))

    identity = singles.tile([P, P], BF16)
    make_identity(nc, identity)
    ones = singles.tile([P, 1], BF16)
    nc.gpsimd.memset(ones[:], 1.0)

    # Per-batch exp-rowsums over k (f32). Matmul out base partition must be
    # 0/32/64, so batch b lands in bank b//3 at partition row (b%3)*32.
    rs_banks = [
        rs_pool.tile([P, C], F32, tag=f"rs{i}", name=f"rs{i}") for i in range(3)
    ]
    # init: the final per-bank ln reads the unused rows too; ln(1)=0 keeps
    # them finite (they are discarded by the strided gather anyway).
    # GPSIMD cannot touch PSUM, so these run on DVE (once, off critical path).
    for rb in rs_banks:
        nc.vector.memset(rb[:], 1.0)
    plog_all = singles.tile([P, NCHUNK * B_PER_CORE], F32)

    for pair in range(NPAIR):
        b0 = pair * PB
        # ---- load a pair of batches (cast f32 -> bf16 in the DMA) ----
        V = inputs.tile([P, PB, NCHUNK, D], BF16, tag="V")
        T = inputs.tile([P, PB, NCHUNK, D], BF16, tag="T")
        nc.gpsimd.dma_start(
            out=V[:], in_=v_ap[b0 : b0 + PB].rearrange("b (n p) d -> p b n d", p=P)
        )
        nc.gpsimd.dma_start(
            out=T[:], in_=t_ap[b0 : b0 + PB].rearrange("b (n p) d -> p b n d", p=P)
        )
        Vr = V[:].rearrange("p b n d -> p (b n) d")
        Tr = T[:].rearrange("p b n d -> p (b n) d")

        # ---- fused elementwise stage: [v*v | t*t | v*t] -> [128, 24, 256] --
        GPC = PB * NCHUNK  # 8 groups per product class
        sq = sqpool.tile([P, NG, D], BF16, tag="sq")
        nc.vector.tensor_mul(sq[:, 0:GPC, :], Vr, Vr)
        nc.vector.tensor_mul(sq[:, GPC : 2 * GPC, :], Tr, Tr)
        nc.vector.tensor_mul(sq[:, 2 * GPC : NG, :], Vr, Tr)

        # halving tree (2x-mode adds) then grouped 1x reduce
        h1 = hpool.tile([P, NG, D // 2], BF16, tag="h1")
        nc.vector.tensor_add(h1[:], sq[:, :, 0 : D // 2], sq[:, :, D // 2 : D])
        h2 = hpool.tile([P, NG, D // 4], BF16, tag="h2")
        nc.vector.tensor_add(h2[:], h1[:, :, 0 : D // 4], h1[:, :, D // 4 : D // 2])
        nvt = stats.tile([P, NG], BF16, tag="nvt")
        nc.vector.reduce_sum(nvt[:], h2[:], axis=mybir.AxisListType.X)

        # ---- scales on ACT (single table set):
        #   scl[:,0:8]  = exp(-0.5*ln(0.25*nv2)) = 2/||v||   (temp folded)
        #   scl[:,8:16] = exp(-0.5*ln(nt2))      = 1/||t||
        lnall = stats.tile([P, 2 * GPC], F32, tag="lnall")
        scl = stats.tile([P, 2 * GPC], F32, tag="scl")
        nc.scalar.activation(
            lnall[:, 0:GPC],
            nvt[:, 0:GPC],
            mybir.ActivationFunctionType.Ln,
            scale=0.25,
        )
        nc.scalar.activation(
            lnall[:, GPC : 2 * GPC],
            nvt[:, GPC : 2 * GPC],
            mybir.ActivationFunctionType.Ln,
        )
        nc.scalar.activation(
            scl[:], lnall[:], mybir.ActivationFunctionType.Exp, scale=-0.5
        )

        # ---- positive logits: plog = (v.t) * sv * tsc ----
        svtsc = stats.tile([P, GPC], F32, tag="svtsc")
        nc.vector.tensor_mul(svtsc[:], scl[:, 0:GPC], scl[:, GPC : 2 * GPC])
        nc.vector.tensor_mul(
            plog_all[:, b0 * NCHUNK : (b0 + PB) * NCHUNK],
            nvt[:, 2 * GPC : NG],
            svtsc[:],
        )

        # ---- Vhat = V * sv  (one op; sv broadcast along d via stride-0) ----
        Vh = normed.tile([P, PB, NCHUNK, D], BF16, tag="Vh")
        sv_bcast = _bcast_cols(scl, 0, PB, NCHUNK, D)
        nc.vector.tensor_tensor(
            out=Vh[:], in0=V[:], in1=sv_bcast, op=mybir.AluOpType.mult
        )

        # ---- per batch: transpose Vhat & raw T; Gram; exp; PE rowsum ----
        for pb in range(PB):
            b = b0 + pb
            Vt = trans.tile([P, NDHALF, C], BF16, tag="Vt")
            tpv = tp_pool.tile([P, NDHALF, C], BF16, tag="tp")
            for e in range(NDHALF):
                for j in range(NCHUNK):
                    nc.tensor.transpose(
                        tpv[:, e, j * P : (j + 1) * P],
                        Vh[:, pb, j, e * P : (e + 1) * P],
                        identity,
                    )
            nc.scalar.activation(
                Vt[:], tpv[:], mybir.ActivationFunctionType.Copy
            )

            Tt = trans.tile([P, NDHALF, C], BF16, tag="Tt")
            tpt = tp_pool.tile([P, NDHALF, C], BF16, tag="tp")
            for e in range(NDHALF):
                for j in range(NCHUNK):
                    nc.tensor.transpose(
                        tpt[:, e, j * P : (j + 1) * P],
                        T[:, pb, j, e * P : (e + 1) * P],
                        identity,
                    )
            nc.scalar.activation(
                Tt[:], tpt[:], mybir.ActivationFunctionType.Copy
            )

            for j in range(NCHUNK):
                gp = gp_pool.tile([P, C], F32, tag="gp")
                nc.tensor.matmul(
                    gp[:],
                    lhsT=Tt[:, 0, j * P : (j + 1) * P],
                    rhs=Vt[:, 0, :],
                    start=True,
                    stop=False,
                )
                nc.tensor.matmul(
                    gp[:],
                    lhsT=Tt[:, 1, j * P : (j + 1) * P],
                    rhs=Vt[:, 1, :],
                    start=False,
                    stop=True,
                )
                # E'[k-block j, c] = exp(tsc_k * (t_k . vhat_c))
                E = epool.tile([P, C], BF16, tag="E")
                nc.scalar.activation(
                    E[:],
                    gp[:],
                    mybir.ActivationFunctionType.Exp,
                    scale=scl[:, GPC + pb * NCHUNK + j : GPC + pb * NCHUNK + j + 1],
                )
                # rowsum over k (partition dim here): ones-matmul into the
                # batch's PSUM row (bank b//3, base partition (b%3)*32)
                row = (b % 3) * 32
                nc.tensor.matmul(
                    rs_banks[b // 3][row : row + 1, :],
                    lhsT=ones[:],
                    rhs=E[:],
                    start=(j == 0),
                    stop=(j == NCHUNK - 1),
                )

    # ---- finals: per bank, ln(PSUM)->SBUF, row-reduce, then DMA-gather the
    # valid (stride-32) partition rows straight into the DRAM output ----
    lossB = singles.tile([P, 1], F32)
    nc.vector.reduce_sum(lossB[:], plog_all[:], axis=mybir.AxisListType.X)
    nc.sync.dma_start(out=lossB_ap, in_=lossB[:])
    for i in range(3):
        rows = [b for b in range(B_PER_CORE) if b // 3 == i]
        hi = (len(rows) - 1) * 32 + 1  # highest valid partition row + 1
        lnr_i = singles.tile([hi, C], F32, name=f"lnr{i}")
        nc.scalar.activation(
            lnr_i[:], rs_banks[i][0:hi, :], mybir.ActivationFunctionType.Ln
        )
        red_i = singles.tile([hi, 1], F32, name=f"red{i}")
        nc.vector.reduce_sum(red_i[:], lnr_i[:], axis=mybir.AxisListType.X)
        src = red_i[:]
        gather = bass.AP(
            tensor=src.tensor,
            offset=src.offset,
            ap=[[32, len(rows)], [1, 1]],
        )
        nc.sync.dma_start(
            out=lossA_ap[rows[0] : rows[0] + len(rows)], in_=gather
        )


_NC_CACHE = []


def _get_nc():
    if not _NC_CACHE:
        nc = bacc.Bacc("TRN2", target_bir_lowering=False, debug=False)
        v_dram = nc.dram_tensor("v", [B_PER_CORE, C, D], F32, kind="ExternalInput")
        t_dram = nc.dram_tensor("t", [B_PER_CORE, C, D], F32, kind="ExternalInput")
        lossA_dram = nc.dram_tensor(
            "lossA", [B_PER_CORE, 1], F32, kind="ExternalOutput"
        )
        lossB_dram = nc.dram_tensor("lossB", [P, 1], F32, kind="ExternalOutput")
        with tile.TileContext(nc) as tc, ExitStack() as ctx:
            _emit(ctx, tc, lossA_dram.ap(), lossB_dram.ap(), v_dram.ap(), t_dram.ap())
        nc.compile()
        _NC_CACHE.append(nc)
    return _NC_CACHE[0]


def kernel(visual_features, text_embeddings):
    v = np.ascontiguousarray(np.asarray(visual_features, dtype=np.float32))
    t = np.ascontiguousarray(np.asarray(text_embeddings, dtype=np.float32))
    v = v.reshape(N_CORES, B_PER_CORE, C, D)
    t = t.reshape(N_CORES, B_PER_CORE, C, D)
    in_maps = [{"v": v[i], "t": t[i]} for i in range(N_CORES)]
    nc = _get_nc()
    res = run_bass_kernel_spmd(nc, in_maps, list(range(N_CORES)))
    total = 0.0
    for r in res.results:
        total += float(r["lossA"].astype(np.float64).sum())
        total -= float(r["lossB"].astype(np.float64).sum())
    return np.float32(total / (N_CORES * B_PER_CORE * C))


# revision 14
# speedup vs baseline: 1.0234x; 1.0234x over previous
"""Trainium2 Bass kernel for batched contrastive loss (InfoNCE over CxC sims).

Math (matches the jax reference):
    v_hat = v / ||v||,  t_hat = t / ||t||          (L2 over D, eps=1e-12)
    L[b,c,k] = (v_hat[b,c] . t_hat[b,k]) / 0.5     (logits)
    loss = mean_{b,c} [ logsumexp_k L[b,c,k] - L[b,c,c] ]

Strategy (8 NeuronCores, data-parallel over B=64 -> 8 batches/core).

v2 rebalance (from v1 trace: Vector 62us / Scalar 57us busy of an 89us span):
  - ONE fused elementwise stage: v*v, t*t and v*t all write one concatenated
    scratch [128, 24, 256]; a two-level halving tree (2x-mode tensor_adds)
    shrinks it 4x before the 1x-mode grouped reduce. Norms AND the positive
    dot products come out of a single [128, 24] reduce per pair.
  - Transposed Gram G'[k, c] = (Tt)^T @ Vhat_t: V is pre-scaled by
    sv=2/||v|| (per-partition bcast mult in natural layout), so the exp's
    per-partition scale slot carries tsc_k = 1/||t_k||. No T_hat tensor.
  - Row-sums over k are PARTITION sums in this orientation: a ones[128,1]
    matmul accumulates exp chunks into one PSUM bank (rows = batches).
    Kills all 32 ACT accumulator reads (~9.3us) of v1.
  - PSUM->SBUF transpose copies moved off Scalar to GpSimd/Pool (copy
    efficiency 0.6 vs multiply 0.42 -> copies are the one big op class
    Pool does ~as well as the busy engines).
  - rsqrt without Rsqrt and within ONE ACT table set (via the
    get_activation_tables patch): sv = exp(-0.5*ln(nv2/4)), tsc =
    exp(-0.5*ln(nt2)).
  - Output: per-core partial sums lossA[8,1] = sum_c ln(rowsum) per batch,
    lossB[128,1] = sum of positive logits; host combines.
"""

import math
from contextlib import ExitStack

import numpy as np

import concourse.bacc as bacc
import concourse.bass as bass
import concourse.tile as tile
from concourse import mybir
from concourse.bass_utils import run_bass_kernel_spmd
from concourse.masks import make_identity

N_CORES = 8
B_PER_CORE = 8
PB = 2  # batches per DMA pair
NPAIR = B_PER_CORE // PB
C = 512
D = 256
P = 128
NCHUNK = C // P  # 4 c-chunks per batch
NDHALF = D // P  # 2 d-halves
NG = 3 * PB * NCHUNK  # 24 reduce groups: [v*v | t*t | v*t] x (pb, chunk)

F32 = mybir.dt.float32
BF16 = mybir.dt.bfloat16

# ---------------------------------------------------------------------------
# Keep ACT on a single table set: exp & ln both live in
# "natural_log_exp_and_others"; by removing them from every other set, the
# insert_act_table_loads fixpoint must pick that one set for both, so the
# kernel pays ONE table load instead of thrashing (~1.3us per reload).
_orig_get_tables = bacc.get_activation_tables


def _patched_get_tables(arch):
    tables = dict(_orig_get_tables(arch))
    keep = "natural_log_exp_and_others"
    strip = {mybir.ActivationFunctionType.Exp, mybir.ActivationFunctionType.Ln}
    if keep in tables:
        for name in tables:
            if name != keep:
                tables[name] = set(tables[name]) - strip
    return tables


bacc.get_activation_tables = _patched_get_tables


def _bcast_cols(tile_ap, col0, ncols_outer, ncols_inner, bcast_count):
    """AP reading tile[:, col0 + o*ncols_inner + i] broadcast bcast_count
    times along a new innermost (stride-0) dim."""
    base = tile_ap[:, col0 : col0 + ncols_outer * ncols_inner]
    part_dim = base.ap[0]
    elem_step = base.ap[-1][0]
    return bass.AP(
        tensor=base.tensor,
        offset=base.offset,
        ap=[
            part_dim,
            [elem_step * ncols_inner, ncols_outer],
            [elem_step, ncols_inner],
            [0, bcast_count],
        ],
    )


def _emit(ctx: ExitStack, tc: tile.TileContext, lossA_ap, lossB_ap, v_ap, t_ap):
    nc = tc.nc
    ctx.enter_context(nc.allow_low_precision("bf16 norm/pos reduces, 2x DVE"))

    singles = ctx.enter_context(tc.tile_pool(name="singles", bufs=1))
    inputs = ctx.enter_context(tc.tile_pool(name="inputs", bufs=4))
    normed = ctx.enter_context(tc.tile_pool(name="normed", bufs=2))
    sqpool = ctx.enter_context(tc.tile_pool(name="sq", bufs=2))
    hpool = ctx.enter_context(tc.tile_pool(name="h", bufs=2))
    trans = ctx.enter_context(tc.tile_pool(name="trans", bufs=4))
    epool = ctx.enter_context(tc.tile_pool(name="E", bufs=6))
    stats = ctx.enter_context(tc.tile_pool(name="stats", bufs=2))
    tp_pool = ctx.enter_context(tc.tile_pool(name="tp", bufs=3, space="PSUM"))
    gp_pool = ctx.enter_context(tc.tile_pool(name="gp", bufs=2, space="PSUM"))
    rs_pool = ctx.enter_context(tc.tile_pool(name="rs", bufs=1, space="PSUM"))

    # ---- ALL input loads first: the SWDGE issue ops head the Pool queue so
    # DMA streams from t~0 (identity/memsets used to delay this by ~6us) ----
    V_tiles, T_tiles = [], []
    for pair in range(NPAIR):
        b0 = pair * PB
        V = inputs.tile([P, PB, NCHUNK, D], BF16, tag="V", name=f"V{pair}")
        T = inputs.tile([P, PB, NCHUNK, D], BF16, tag="T", name=f"T{pair}")
        nc.gpsimd.dma_start(out=V[:], in_=v_ap[:, b0 : b0 + PB])
        nc.gpsimd.dma_start(out=T[:], in_=t_ap[:, b0 : b0 + PB])
        V_tiles.append(V)
        T_tiles.append(T)

    identity = singles.tile([P, P], BF16)
    make_identity(nc, identity)
    ones = singles.tile([P, 1], BF16)
    nc.vector.memset(ones[:], 1.0)

    # Per-batch exp-rowsums over k (f32). Matmul out base partition must be
    # 0/32/64, so batch b lands in bank b//3 at partition row (b%3)*32.
    rs_banks = [
        rs_pool.tile([P, C], F32, tag=f"rs{i}", name=f"rs{i}") for i in range(3)
    ]
    for rb in rs_banks:
        nc.vector.memset(rb[:], 1.0)
    plog_all = singles.tile([P, NCHUNK * B_PER_CORE], F32)

    GPC = PB * NCHUNK  # 8 groups per product class
    pending = None  # (batch, [E chunks]) rowsums delayed one batch (PE never
    #                 waits on ACT: next batch's Grams run first)

    def emit_rowsums(b, E_list):
        for j in range(NCHUNK):
            row = (b % 3) * 32
            nc.tensor.matmul(
                rs_banks[b // 3][row : row + 1, :],
                lhsT=ones[:],
                rhs=E_list[j][:],
                start=(j == 0),
                stop=(j == NCHUNK - 1),
            )
        # bank complete -> finals overlap the remaining batches
        if b in (2, 5, 7):
            i = b // 3
            rows = [x for x in range(B_PER_CORE) if x // 3 == i]
            hi = (len(rows) - 1) * 32 + 1
            lnr_i = singles.tile([hi, C], F32, name=f"lnr{i}")
            nc.scalar.activation(
                lnr_i[:], rs_banks[i][0:hi, :], mybir.ActivationFunctionType.Ln
            )
            red_i = singles.tile([hi, 1], F32, name=f"red{i}")
            nc.vector.reduce_sum(red_i[:], lnr_i[:], axis=mybir.AxisListType.X)
            src = red_i[:]
            gather = bass.AP(
                tensor=src.tensor,
                offset=src.offset,
                ap=[[32, len(rows)], [1, 1]],
            )
            nc.sync.dma_start(
                out=lossA_ap[rows[0] : rows[0] + len(rows)], in_=gather
            )

    for pair in range(NPAIR):
        b0 = pair * PB
        V, T = V_tiles[pair], T_tiles[pair]
        Vr = V[:].rearrange("p b n d -> p (b n) d")
        Tr = T[:].rearrange("p b n d -> p (b n) d")

        # ---- fused elementwise stage: [v*v | t*t | v*t] -> [128, 24, 256] --
        sq = sqpool.tile([P, NG, D], BF16, tag="sq")
        nc.vector.tensor_mul(sq[:, 0:GPC, :], Vr, Vr)
        nc.vector.tensor_mul(sq[:, GPC : 2 * GPC, :], Tr, Tr)
        nc.vector.tensor_mul(sq[:, 2 * GPC : NG, :], Vr, Tr)

        # halving tree (2x-mode adds) then grouped 1x reduce
        h1 = hpool.tile([P, NG, D // 2], BF16, tag="h1")
        nc.vector.tensor_add(h1[:], sq[:, :, 0 : D // 2], sq[:, :, D // 2 : D])
        h2 = hpool.tile([P, NG, D // 4], BF16, tag="h2")
        nc.vector.tensor_add(h2[:], h1[:, :, 0 : D // 4], h1[:, :, D // 4 : D // 2])
        nvt = stats.tile([P, NG], BF16, tag="nvt")
        nc.vector.reduce_sum(nvt[:], h2[:], axis=mybir.AxisListType.X)

        # ---- scales on ACT (single table set):
        #   scl[:,0:8]  = exp(-0.5*ln(0.25*nv2)) = 2/||v||   (temp folded)
        #   scl[:,8:16] = exp(-0.5*ln(nt2))      = 1/||t||
        lnall = stats.tile([P, 2 * GPC], F32, tag="lnall")
        scl = stats.tile([P, 2 * GPC], F32, tag="scl")
        nc.scalar.activation(
            lnall[:, 0:GPC],
            nvt[:, 0:GPC],
            mybir.ActivationFunctionType.Ln,
            scale=0.25,
        )
        nc.scalar.activation(
            lnall[:, GPC : 2 * GPC],
            nvt[:, GPC : 2 * GPC],
            mybir.ActivationFunctionType.Ln,
        )
        nc.scalar.activation(
            scl[:], lnall[:], mybir.ActivationFunctionType.Exp, scale=-0.5
        )

        # ---- positive logits: plog = (v.t) * sv * tsc ----
        svtsc = stats.tile([P, GPC], F32, tag="svtsc")
        nc.vector.tensor_mul(svtsc[:], scl[:, 0:GPC], scl[:, GPC : 2 * GPC])
        nc.vector.tensor_mul(
            plog_all[:, b0 * NCHUNK : (b0 + PB) * NCHUNK],
            nvt[:, 2 * GPC : NG],
            svtsc[:],
        )

        # ---- Vhat = V * sv  (one op; sv broadcast along d via stride-0) ----
        Vh = normed.tile([P, PB, NCHUNK, D], BF16, tag="Vh")
        sv_bcast = _bcast_cols(scl, 0, PB, NCHUNK, D)
        nc.vector.tensor_tensor(
            out=Vh[:], in0=V[:], in1=sv_bcast, op=mybir.AluOpType.mult
        )

        # ---- per batch: transpose Vhat & raw T; Gram; exp; PE rowsum ----
        for pb in range(PB):
            b = b0 + pb
            Vt = trans.tile([P, NDHALF, C], BF16, tag="Vt")
            tpv = tp_pool.tile([P, NDHALF, C], BF16, tag="tp")
            for e in range(NDHALF):
                for j in range(NCHUNK):
                    nc.tensor.transpose(
                        tpv[:, e, j * P : (j + 1) * P],
                        Vh[:, pb, j, e * P : (e + 1) * P],
                        identity,
                    )
            nc.scalar.activation(
                Vt[:], tpv[:], mybir.ActivationFunctionType.Copy
            )

            Tt = trans.tile([P, NDHALF, C], BF16, tag="Tt")
            tpt = tp_pool.tile([P, NDHALF, C], BF16, tag="tp")
            for e in range(NDHALF):
                for j in range(NCHUNK):
                    nc.tensor.transpose(
                        tpt[:, e, j * P : (j + 1) * P],
                        T[:, pb, j, e * P : (e + 1) * P],
                        identity,
                    )
            nc.scalar.activation(
                Tt[:], tpt[:], mybir.ActivationFunctionType.Copy
            )

            gps = []
            for j in range(NCHUNK):
                gp = gp_pool.tile([P, C], F32, tag="gp")
                nc.tensor.matmul(
                    gp[:],
                    lhsT=Tt[:, 0, j * P : (j + 1) * P],
                    rhs=Vt[:, 0, :],
                    start=True,
                    stop=False,
                )
                nc.tensor.matmul(
                    gp[:],
                    lhsT=Tt[:, 1, j * P : (j + 1) * P],
                    rhs=Vt[:, 1, :],
                    start=False,
                    stop=True,
                )
                gps.append(gp)

            # previous batch's rowsums AFTER this batch's Grams: when the PE
            # reaches them, ACT has long since produced those E chunks
            if pending is not None:
                emit_rowsums(*pending)

            E_list = []
            for j in range(NCHUNK):
                # E'[k-block j, c] = exp(tsc_k * (t_k . vhat_c))
                E = epool.tile([P, C], BF16, tag="E")
                nc.scalar.activation(
                    E[:],
                    gps[j][:],
                    mybir.ActivationFunctionType.Exp,
                    scale=scl[:, GPC + pb * NCHUNK + j : GPC + pb * NCHUNK + j + 1],
                )
                E_list.append(E)
            pending = (b, E_list)

    emit_rowsums(*pending)

    lossB = singles.tile([P, 1], F32)
    nc.vector.reduce_sum(lossB[:], plog_all[:], axis=mybir.AxisListType.X)
    nc.sync.dma_start(out=lossB_ap, in_=lossB[:])


_NC_CACHE = []


def _get_nc():
    if not _NC_CACHE:
        nc = bacc.Bacc("TRN2", target_bir_lowering=False, debug=False)
        v_dram = nc.dram_tensor(
            "v", [P, B_PER_CORE, NCHUNK, D], F32, kind="ExternalInput"
        )
        t_dram = nc.dram_tensor(
            "t", [P, B_PER_CORE, NCHUNK, D], F32, kind="ExternalInput"
        )
        lossA_dram = nc.dram_tensor(
            "lossA", [B_PER_CORE, 1], F32, kind="ExternalOutput"
        )
        lossB_dram = nc.dram_tensor("lossB", [P, 1], F32, kind="ExternalOutput")
        with tile.TileContext(nc) as tc, ExitStack() as ctx:
            _emit(ctx, tc, lossA_dram.ap(), lossB_dram.ap(), v_dram.ap(), t_dram.ap())
        nc.compile()
        _NC_CACHE.append(nc)
    return _NC_CACHE[0]


def _pack(x):
    """[N_CORES*B, C, D] f32 -> per-core [P, B, NCHUNK, D]: every partition's
    data is one contiguous 8KB-per-pair run in DRAM (big DMA descriptors)."""
    x = np.asarray(x, dtype=np.float32).reshape(
        N_CORES, B_PER_CORE, NCHUNK, P, D
    )
    return np.ascontiguousarray(x.transpose(0, 3, 1, 2, 4))


def kernel(visual_features, text_embeddings):
    v = _pack(visual_features)
    t = _pack(text_embeddings)
    in_maps = [{"v": v[i], "t": t[i]} for i in range(N_CORES)]
    nc = _get_nc()
    res = run_bass_kernel_spmd(nc, in_maps, list(range(N_CORES)))
    total = 0.0
    for r in res.results:
        total += float(r["lossA"].astype(np.float64).sum())
        total -= float(r["lossB"].astype(np.float64).sum())
    return np.float32(total / (N_CORES * B_PER_CORE * C))


# revision 16
# speedup vs baseline: 1.1142x; 1.0888x over previous
"""Trainium2 Bass kernel for batched contrastive loss (InfoNCE over CxC sims).

Math (matches the jax reference):
    v_hat = v / ||v||,  t_hat = t / ||t||          (L2 over D, eps=1e-12)
    L[b,c,k] = (v_hat[b,c] . t_hat[b,k]) / 0.5     (logits)
    loss = mean_{b,c} [ logsumexp_k L[b,c,k] - L[b,c,c] ]

Strategy (8 NeuronCores, data-parallel over B=64 -> 8 batches/core).

v2 rebalance (from v1 trace: Vector 62us / Scalar 57us busy of an 89us span):
  - ONE fused elementwise stage: v*v, t*t and v*t all write one concatenated
    scratch [128, 24, 256]; a two-level halving tree (2x-mode tensor_adds)
    shrinks it 4x before the 1x-mode grouped reduce. Norms AND the positive
    dot products come out of a single [128, 24] reduce per pair.
  - Transposed Gram G'[k, c] = (Tt)^T @ Vhat_t: V is pre-scaled by
    sv=2/||v|| (per-partition bcast mult in natural layout), so the exp's
    per-partition scale slot carries tsc_k = 1/||t_k||. No T_hat tensor.
  - Row-sums over k are PARTITION sums in this orientation: a ones[128,1]
    matmul accumulates exp chunks into one PSUM bank (rows = batches).
    Kills all 32 ACT accumulator reads (~9.3us) of v1.
  - PSUM->SBUF transpose copies moved off Scalar to GpSimd/Pool (copy
    efficiency 0.6 vs multiply 0.42 -> copies are the one big op class
    Pool does ~as well as the busy engines).
  - rsqrt without Rsqrt and within ONE ACT table set (via the
    get_activation_tables patch): sv = exp(-0.5*ln(nv2/4)), tsc =
    exp(-0.5*ln(nt2)).
  - Output: per-core partial sums lossA[8,1] = sum_c ln(rowsum) per batch,
    lossB[128,1] = sum of positive logits; host combines.
"""

import math
from contextlib import ExitStack

import numpy as np

import concourse.bacc as bacc
import concourse.bass as bass
import concourse.tile as tile
from concourse import mybir
from concourse.bass_utils import run_bass_kernel_spmd
from concourse.masks import make_identity

N_CORES = 8
B_PER_CORE = 8
PB = 2  # batches per DMA pair
NPAIR = B_PER_CORE // PB
C = 512
D = 256
P = 128
NCHUNK = C // P  # 4 c-chunks per batch
NDHALF = D // P  # 2 d-halves
NG = 3 * PB * NCHUNK  # 24 reduce groups: [v*v | t*t | v*t] x (pb, chunk)

F32 = mybir.dt.float32
BF16 = mybir.dt.bfloat16

# ---------------------------------------------------------------------------
# Keep ACT on a single table set: exp & ln both live in
# "natural_log_exp_and_others"; by removing them from every other set, the
# insert_act_table_loads fixpoint must pick that one set for both, so the
# kernel pays ONE table load instead of thrashing (~1.3us per reload).
_orig_get_tables = bacc.get_activation_tables


def _patched_get_tables(arch):
    tables = dict(_orig_get_tables(arch))
    keep = "natural_log_exp_and_others"
    strip = {mybir.ActivationFunctionType.Exp, mybir.ActivationFunctionType.Ln}
    if keep in tables:
        for name in tables:
            if name != keep:
                tables[name] = set(tables[name]) - strip
    return tables


bacc.get_activation_tables = _patched_get_tables


def _bcast_cols(tile_ap, col0, ncols_outer, ncols_inner, bcast_count):
    """AP reading tile[:, col0 + o*ncols_inner + i] broadcast bcast_count
    times along a new innermost (stride-0) dim."""
    base = tile_ap[:, col0 : col0 + ncols_outer * ncols_inner]
    part_dim = base.ap[0]
    elem_step = base.ap[-1][0]
    return bass.AP(
        tensor=base.tensor,
        offset=base.offset,
        ap=[
            part_dim,
            [elem_step * ncols_inner, ncols_outer],
            [elem_step, ncols_inner],
            [0, bcast_count],
        ],
    )


def _emit(ctx: ExitStack, tc: tile.TileContext, lossA_ap, lossB_ap, v_ap, t_ap):
    nc = tc.nc
    ctx.enter_context(nc.allow_low_precision("bf16 norm/pos reduces, 2x DVE"))

    singles = ctx.enter_context(tc.tile_pool(name="singles", bufs=1))
    inputs = ctx.enter_context(tc.tile_pool(name="inputs", bufs=4))
    normed = ctx.enter_context(tc.tile_pool(name="normed", bufs=2))
    sqpool = ctx.enter_context(tc.tile_pool(name="sq", bufs=2))
    hpool = ctx.enter_context(tc.tile_pool(name="h", bufs=2))
    trans = ctx.enter_context(tc.tile_pool(name="trans", bufs=4))
    epool = ctx.enter_context(tc.tile_pool(name="E", bufs=6))
    stats_pool = ctx.enter_context(tc.tile_pool(name="stats", bufs=2))
    tp_pool = ctx.enter_context(tc.tile_pool(name="tp", bufs=3, space="PSUM"))
    gp_pool = ctx.enter_context(tc.tile_pool(name="gp", bufs=2, space="PSUM"))
    rs_pool = ctx.enter_context(tc.tile_pool(name="rs", bufs=1, space="PSUM"))

    # ---- ALL input loads first: the SWDGE issue ops head the Pool queue so
    # DMA streams from t~0 (identity/memsets used to delay this by ~6us) ----
    V_tiles, T_tiles = [], []
    for pair in range(NPAIR):
        b0 = pair * PB
        V = inputs.tile([P, PB, NCHUNK, D], BF16, tag="V", name=f"V{pair}")
        T = inputs.tile([P, PB, NCHUNK, D], BF16, tag="T", name=f"T{pair}")
        nc.gpsimd.dma_start(out=V[:], in_=v_ap[:, b0 : b0 + PB])
        nc.gpsimd.dma_start(out=T[:], in_=t_ap[:, b0 : b0 + PB])
        V_tiles.append(V)
        T_tiles.append(T)

    identity = singles.tile([P, P], BF16)
    make_identity(nc, identity)
    ones = singles.tile([P, 1], BF16)
    nc.vector.memset(ones[:], 1.0)

    # Per-batch exp-rowsums over k (f32). Matmul out base partition must be
    # 0/32/64, so batch b lands in bank b//3 at partition row (b%3)*32.
    rs_banks = [
        rs_pool.tile([P, C], F32, tag=f"rs{i}", name=f"rs{i}") for i in range(3)
    ]
    for rb in rs_banks:
        nc.vector.memset(rb[:], 1.0)
    plog_all = singles.tile([P, NCHUNK * B_PER_CORE], F32)

    GPC = PB * NCHUNK  # 8 groups per product class
    pending = None  # (batch, [E chunks]) rowsums delayed one batch (PE never
    #                 waits on ACT: next batch's Grams run first)

    def emit_rowsums(b, E_list):
        for j in range(NCHUNK):
            row = (b % 3) * 32
            nc.tensor.matmul(
                rs_banks[b // 3][row : row + 1, :],
                lhsT=ones[:],
                rhs=E_list[j][:],
                start=(j == 0),
                stop=(j == NCHUNK - 1),
            )
        # bank complete -> finals overlap the remaining batches
        if b in (2, 5, 7):
            i = b // 3
            rows = [x for x in range(B_PER_CORE) if x // 3 == i]
            hi = (len(rows) - 1) * 32 + 1
            lnr_i = singles.tile([hi, C], F32, name=f"lnr{i}")
            nc.scalar.activation(
                lnr_i[:], rs_banks[i][0:hi, :], mybir.ActivationFunctionType.Ln
            )
            red_i = singles.tile([hi, 1], F32, name=f"red{i}")
            nc.vector.reduce_sum(red_i[:], lnr_i[:], axis=mybir.AxisListType.X)
            src = red_i[:]
            gather = bass.AP(
                tensor=src.tensor,
                offset=src.offset,
                ap=[[32, len(rows)], [1, 1]],
            )
            nc.sync.dma_start(
                out=lossA_ap[rows[0] : rows[0] + len(rows)], in_=gather
            )

    def stats(pair):
        """Per-pair elementwise/norm/scale stage (DVE + 3 small ACT ops).
        Emitted one pair AHEAD of batch processing so the ACT scale ops are
        never queued behind the previous pair's exps."""
        b0 = pair * PB
        V, T = V_tiles[pair], T_tiles[pair]
        Vr = V[:].rearrange("p b n d -> p (b n) d")
        Tr = T[:].rearrange("p b n d -> p (b n) d")

        # fused elementwise stage: [v*v | t*t | v*t] -> [128, 24, 256]
        sq = sqpool.tile([P, NG, D], BF16, tag="sq")
        nc.vector.tensor_mul(sq[:, 0:GPC, :], Vr, Vr)
        nc.vector.tensor_mul(sq[:, GPC : 2 * GPC, :], Tr, Tr)
        nc.vector.tensor_mul(sq[:, 2 * GPC : NG, :], Vr, Tr)

        # halving tree (2x-mode adds) then grouped 1x reduce
        h1 = hpool.tile([P, NG, D // 2], BF16, tag="h1")
        nc.vector.tensor_add(h1[:], sq[:, :, 0 : D // 2], sq[:, :, D // 2 : D])
        h2 = hpool.tile([P, NG, D // 4], BF16, tag="h2")
        nc.vector.tensor_add(h2[:], h1[:, :, 0 : D // 4], h1[:, :, D // 4 : D // 2])
        h3 = hpool.tile([P, NG, D // 8], BF16, tag="h3")
        nc.vector.tensor_add(h3[:], h2[:, :, 0 : D // 8], h2[:, :, D // 8 : D // 4])
        nvt = stats_pool.tile([P, NG], BF16, tag="nvt")
        nc.vector.reduce_sum(nvt[:], h3[:], axis=mybir.AxisListType.X)

        # scales on ACT (single table set):
        #   scl[:,0:8]  = exp(-0.5*ln(0.25*nv2)) = 2/||v||   (temp folded)
        #   scl[:,8:16] = exp(-0.5*ln(nt2))      = 1/||t||
        lnall = stats_pool.tile([P, 2 * GPC], F32, tag="lnall")
        scl = stats_pool.tile([P, 2 * GPC], F32, tag="scl")
        nc.scalar.activation(
            lnall[:, 0:GPC],
            nvt[:, 0:GPC],
            mybir.ActivationFunctionType.Ln,
            scale=0.25,
        )
        nc.scalar.activation(
            lnall[:, GPC : 2 * GPC],
            nvt[:, GPC : 2 * GPC],
            mybir.ActivationFunctionType.Ln,
        )
        nc.scalar.activation(
            scl[:], lnall[:], mybir.ActivationFunctionType.Exp, scale=-0.5
        )

        # positive logits: plog = (v.t) * sv * tsc
        svtsc = stats_pool.tile([P, GPC], F32, tag="svtsc")
        nc.vector.tensor_mul(svtsc[:], scl[:, 0:GPC], scl[:, GPC : 2 * GPC])
        nc.vector.tensor_mul(
            plog_all[:, b0 * NCHUNK : (b0 + PB) * NCHUNK],
            nvt[:, 2 * GPC : NG],
            svtsc[:],
        )

        # Vhat = V * sv  (one op; sv broadcast along d via stride-0)
        Vh = normed.tile([P, PB, NCHUNK, D], BF16, tag="Vh")
        sv_bcast = _bcast_cols(scl, 0, PB, NCHUNK, D)
        nc.vector.tensor_tensor(
            out=Vh[:], in0=V[:], in1=sv_bcast, op=mybir.AluOpType.mult
        )
        return Vh, scl

    def batches(pair, Vh, scl):
        nonlocal pending
        b0 = pair * PB
        T = T_tiles[pair]
        for pb in range(PB):
            b = b0 + pb
            Vt = trans.tile([P, NDHALF, C], BF16, tag="Vt")
            tpv = tp_pool.tile([P, NDHALF, C], BF16, tag="tp")
            for e in range(NDHALF):
                for j in range(NCHUNK):
                    nc.tensor.transpose(
                        tpv[:, e, j * P : (j + 1) * P],
                        Vh[:, pb, j, e * P : (e + 1) * P],
                        identity,
                    )
            nc.scalar.activation(
                Vt[:], tpv[:], mybir.ActivationFunctionType.Copy
            )

            Tt = trans.tile([P, NDHALF, C], BF16, tag="Tt")
            tpt = tp_pool.tile([P, NDHALF, C], BF16, tag="tp")
            for e in range(NDHALF):
                for j in range(NCHUNK):
                    nc.tensor.transpose(
                        tpt[:, e, j * P : (j + 1) * P],
                        T[:, pb, j, e * P : (e + 1) * P],
                        identity,
                    )
            nc.scalar.activation(
                Tt[:], tpt[:], mybir.ActivationFunctionType.Copy
            )

            gps = []
            for j in range(NCHUNK):
                gp = gp_pool.tile([P, C], F32, tag="gp")
                nc.tensor.matmul(
                    gp[:],
                    lhsT=Tt[:, 0, j * P : (j + 1) * P],
                    rhs=Vt[:, 0, :],
                    start=True,
                    stop=False,
                )
                nc.tensor.matmul(
                    gp[:],
                    lhsT=Tt[:, 1, j * P : (j + 1) * P],
                    rhs=Vt[:, 1, :],
                    start=False,
                    stop=True,
                )
                gps.append(gp)

            # previous batch's rowsums AFTER this batch's Grams: when the PE
            # reaches them, ACT has long since produced those E chunks
            if pending is not None:
                emit_rowsums(*pending)

            E_list = []
            for j in range(NCHUNK):
                # E'[k-block j, c] = exp(tsc_k * (t_k . vhat_c))
                E = epool.tile([P, C], BF16, tag="E")
                nc.scalar.activation(
                    E[:],
                    gps[j][:],
                    mybir.ActivationFunctionType.Exp,
                    scale=scl[:, GPC + pb * NCHUNK + j : GPC + pb * NCHUNK + j + 1],
                )
                E_list.append(E)
            pending = (b, E_list)

    carry = None
    for pair in range(NPAIR):
        Vh, scl = stats(pair)
        if carry is not None:
            batches(*carry)
        carry = (pair, Vh, scl)
    batches(*carry)
    emit_rowsums(*pending)

    lossB = singles.tile([P, 1], F32)
    nc.vector.reduce_sum(lossB[:], plog_all[:], axis=mybir.AxisListType.X)
    nc.sync.dma_start(out=lossB_ap, in_=lossB[:])


_NC_CACHE = []


def _get_nc():
    if not _NC_CACHE:
        nc = bacc.Bacc("TRN2", target_bir_lowering=False, debug=False)
        v_dram = nc.dram_tensor(
            "v", [P, B_PER_CORE, NCHUNK, D], F32, kind="ExternalInput"
        )
        t_dram = nc.dram_tensor(
            "t", [P, B_PER_CORE, NCHUNK, D], F32, kind="ExternalInput"
        )
        lossA_dram = nc.dram_tensor(
            "lossA", [B_PER_CORE, 1], F32, kind="ExternalOutput"
        )
        lossB_dram = nc.dram_tensor("lossB", [P, 1], F32, kind="ExternalOutput")
        with tile.TileContext(nc) as tc, ExitStack() as ctx:
            _emit(ctx, tc, lossA_dram.ap(), lossB_dram.ap(), v_dram.ap(), t_dram.ap())
        nc.compile()
        _NC_CACHE.append(nc)
    return _NC_CACHE[0]


def _pack(x):
    """[N_CORES*B, C, D] f32 -> per-core [P, B, NCHUNK, D]: every partition's
    data is one contiguous 8KB-per-pair run in DRAM (big DMA descriptors)."""
    x = np.asarray(x, dtype=np.float32).reshape(
        N_CORES, B_PER_CORE, NCHUNK, P, D
    )
    return np.ascontiguousarray(x.transpose(0, 3, 1, 2, 4))


def kernel(visual_features, text_embeddings):
    v = _pack(visual_features)
    t = _pack(text_embeddings)
    in_maps = [{"v": v[i], "t": t[i]} for i in range(N_CORES)]
    nc = _get_nc()
    res = run_bass_kernel_spmd(nc, in_maps, list(range(N_CORES)))
    total = 0.0
    for r in res.results:
        total += float(r["lossA"].astype(np.float64).sum())
        total -= float(r["lossB"].astype(np.float64).sum())
    return np.float32(total / (N_CORES * B_PER_CORE * C))
